# revision 10
# baseline (speedup 1.0000x reference)
"""MaxMarginLoss Trainium2 kernel (8 NeuronCores, vocab-sharded).

Math (reference):
    out_norm = l2norm(preds^T over D)            [B,S,D]
    voc_norm = l2norm(emb over D)                [V,D]
    tgt      = emb[target]                       [B,S,D]
    d        = out_norm@voc_norm.T - tgt@voc_norm.T
    jmax     = argmax_v d
    loss     = mean_masked(relu(g + cos[jmax] - cos[target]))

Key identity: d = (out_norm - tgt) @ voc_norm.T  -> ONE matmul.  Per-row
positive scaling keeps the argmax, so each device computes
    slab[s,v] = (preds[s] - n_s*tgt[s]) . voc_norm[v]   ( = n_s * d[s,v] )
with NO division on device.  The matmul runs in fp8e4m3 (DoubleRow perf
mode, 2 k-subtiles per instruction) accumulating f32 in PSUM.

Device outputs ONLY per-row block maxes (blocks of W=16 vocab columns).
Per 2048-col PSUM half: DVE reduce_max's the first 128 cols straight from
PSUM; the scalar engine copies the other 1920 cols to SBUF as bf16, which
DVE then folds 16->8->4 with tensor_max (2-byte dtype = 2x_1port DVE
speed) and reduce_max's to block maxes.  This balances Act/DVE/PE at
~1.7us per half.  No DRAM slab, no gathers, no argmax scan on device.
(gpsimd is idle: walrus implements TensorTensor mult but not max on Pool,
and Pool cannot read PSUM.)

Host combine picks the winning (core, block) per row from the 8x256 block
maxes, recomputes that block's 16 exact dots in f64-free numpy (33 MFLOP,
same scale as the cos_target dots it already does), resolves the exact
within-block argmax, and finishes the masked-mean loss.
"""

import os
import sys

import numpy as np

for _p in ("/opt/trn_rl_repo", "/root/.axon_site/_ro/trn_rl_repo"):
    if os.path.isdir(_p) and _p not in sys.path:
        sys.path.insert(0, _p)

import concourse.bass as bass
import concourse.bacc as bacc_mod
import concourse.mybir as mybir
from concourse.tile import TileContext

P = 128
B, S, D, V = 4, 512, 512, 32000
BS = B * S                  # 2048 rows
NCORES = 8
VS = V // NCORES            # 4000 vocab rows per core
VSP = 4096                  # padded vocab per core
KC = D // P                 # 4 k-subtiles of the contraction
NT = BS // P                # 16 row tiles
HALF = 2048                 # columns per PSUM half
W = 16                      # block width for block maxes
NBT = VSP // W              # 256 blocks per row tile
NBH = HALF // W             # 128 blocks per half
DVE_COLS = 384              # per-half columns reduced by DVE from PSUM
DVE_NB = DVE_COLS // W      # 24
ACT_COLS = HALF - DVE_COLS  # 1664 copied by Act to bf16, folded by DVE
ACT_NB = ACT_COLS // W      # 104
NVALID = VS // W            # 250 non-pad blocks per core
SCALE_E = 0.125
SCALE_V = 16.0
GAMMA = 0.5

F32 = mybir.dt.float32
BF16 = mybir.dt.bfloat16
F8 = mybir.dt.float8e4

_CACHED = {}


def build_nc():
    nc = bacc_mod.Bacc()

    eT8 = nc.declare_dram_parameter("eT8", [P, KC * BS], F8, isOutput=False)
    voc8 = nc.declare_dram_parameter("voc8", [P, KC * VSP], F8, isOutput=False)
    o_bm = nc.declare_dram_parameter("o_bm", [P, NT * NBT], F32, isOutput=True)

    with TileContext(nc) as tc:
        with (
            tc.tile_pool(name="const", bufs=1) as cpool,
            tc.tile_pool(name="stage", bufs=3) as stage,
            tc.tile_pool(name="foldp", bufs=3) as foldp,
            tc.tile_pool(name="bmp", bufs=3) as bmp,
            tc.tile_pool(name="psp", bufs=2, space="PSUM") as psp,
        ):
            eSB = cpool.tile([P, KC * BS], F8, tag="eSB")
            vSB = cpool.tile([P, KC * VSP], F8, tag="vSB")

            # PE warm-up burst (p-state ramp) while the input DMAs fly
            w0 = cpool.tile([P, 256], F8, tag="w0")
            x0 = cpool.tile([P, 1024], F8, tag="x0")
            nc.vector.memset(w0, 0.0)
            nc.vector.memset(x0, 0.0)
            psw = psp.tile([P, 2048], F32, tag="ps", name="ps_warm")
            for i in range(8):
                nc.tensor.matmul(
                    psw[:, :512],
                    lhsT=w0[:].rearrange("p (k m) -> p k m", k=2),
                    rhs=x0[:].rearrange("p (k m) -> p k m", k=2),
                    start=True, stop=True,
                    perf_mode=mybir.MatmulPerfMode.DoubleRow,
                )

            ev = eSB[:].rearrange("p (k m) -> p k m", k=KC)    # [128,4,2048]
            vv = vSB[:].rearrange("p (k m) -> p k m", k=KC)    # [128,4,4096]
            vdram = voc8[:, :].rearrange("p (k m) -> p k m", k=KC)

            # eT on its own queue; voc in per-chunk loads issued in
            # consumption order so the first matmuls start ~3.5us in.
            nc.sync.dma_start(eSB, eT8[:, :])
            qeng = [nc.scalar, nc.gpsimd]
            for i in range(8):
                off = (i // 4) * HALF + (i % 4) * 512
                qeng[i % 2].dma_start(vv[:, :, off:off + 512],
                                      vdram[:, :, off:off + 512])

            for t in range(NT):
                ts = slice(t * P, (t + 1) * P)
                bmt = bmp.tile([P, NBT], F32, tag="bmt")
                for h in range(2):
                    ps = psp.tile([P, HALF], F32, tag="ps")
                    # q outer: the stationary operand changes only twice per
                    # half, so LDWEIGHTS traffic drops 4x.
                    for q in range(2):
                        for c in range(4):
                            cs = slice(c * 512, (c + 1) * 512)
                            nc.tensor.matmul(
                                ps[:, cs],
                                lhsT=ev[:, 2 * q:2 * q + 2, ts],
                                rhs=vv[:, 2 * q:2 * q + 2,
                                       h * HALF + c * 512:h * HALF + (c + 1) * 512],
                                start=(q == 0), stop=(q == 1),
                                perf_mode=mybir.MatmulPerfMode.DoubleRow,
                            )
                    # DVE: block maxes of the first 128 cols, straight from PSUM
                    nc.vector.reduce_max(
                        bmt[:, h * NBH:h * NBH + DVE_NB],
                        ps[:, 0:DVE_COLS].rearrange("p (b w) -> p b w", w=W),
                        axis=mybir.AxisListType.X,
                    )
                    # Act: bf16-copy remaining 1920 cols; DVE folds 16->8->4
                    # with tensor_max at 2x_1port, then reduce_max's to bm.
                    stg = stage.tile([P, ACT_COLS], BF16, tag="stg")
                    nc.scalar.copy(stg, ps[:, DVE_COLS:HALF])
                    sv = stg[:].rearrange("p (b w) -> p b w", w=W)
                    fs = foldp.tile([P, ACT_NB * 12], BF16, tag="fs")
                    a8 = fs[:, 0:ACT_NB * 8].rearrange("p (b w) -> p b w", w=8)
                    a4 = fs[:, ACT_NB * 8:ACT_NB * 12].rearrange(
                        "p (b w) -> p b w", w=4)
                    nc.vector.tensor_max(a8, sv[:, :, 0:8], sv[:, :, 8:16])
                    nc.vector.tensor_max(a4, a8[:, :, 0:4], a8[:, :, 4:8])
                    nc.vector.reduce_max(
                        bmt[:, h * NBH + DVE_NB:(h + 1) * NBH], a4,
                        axis=mybir.AxisListType.X)
                nc.sync.dma_start(o_bm[:, t * NBT:(t + 1) * NBT], bmt)

    return nc


def get_nc():
    if "nc" not in _CACHED:
        _CACHED["nc"] = build_nc()
    return _CACHED["nc"]


def _prep(preds, emb_weight, target):
    preds = np.ascontiguousarray(np.asarray(preds, dtype=np.float32))     # [B,D,S]
    emb = np.ascontiguousarray(np.asarray(emb_weight, dtype=np.float32))  # [V,D]
    tgt_idx = np.asarray(target).astype(np.int64).reshape(-1)             # [BS]

    predsN = np.ascontiguousarray(preds.transpose(0, 2, 1).reshape(BS, D))
    n = np.maximum(np.sqrt((predsN ** 2).sum(axis=1)), 1e-12).astype(np.float32)
    tgtN = emb[tgt_idx]                                                   # [BS,D]
    er = predsN - n[:, None] * tgtN                                       # [BS,D]
    vocn = emb / np.maximum(
        np.sqrt((emb ** 2).sum(axis=1, keepdims=True)), 1e-12)            # [V,D]
    return predsN, n, tgtN, er, vocn, tgt_idx


def make_in_maps(preds, emb_weight, target):
    import ml_dtypes
    _, _, _, er, vocn, _ = _prep(preds, emb_weight, target)

    e8 = ((er.T) * SCALE_E).astype(ml_dtypes.float8_e4m3)                 # [D,BS]
    eT8 = np.ascontiguousarray(
        e8.reshape(KC, P, BS).transpose(1, 0, 2).reshape(P, KC * BS))

    in_maps = []
    for c in range(NCORES):
        sh = np.zeros((VSP, D), np.float32)
        sh[:VS] = vocn[c * VS:(c + 1) * VS]
        v8 = (sh.T * SCALE_V).astype(ml_dtypes.float8_e4m3)               # [D,VSP]
        voc8 = np.ascontiguousarray(
            v8.reshape(KC, P, VSP).transpose(1, 0, 2).reshape(P, KC * VSP))
        in_maps.append({"eT8": eT8, "voc8": voc8})
    return in_maps


def combine(results, preds, emb_weight, target, pad_id):
    predsN, n, tgtN, er, vocn, tgt_idx = _prep(preds, emb_weight, target)

    # [8, P, NT*NBT] -> M[row, core*NBT + block], row j = t*128 + p
    bm = np.stack([np.asarray(r["o_bm"]) for r in results])
    M = bm.reshape(NCORES, P, NT, NBT).transpose(2, 1, 0, 3).reshape(
        BS, NCORES * NBT)
    pad_mask = np.tile(np.arange(NBT) >= NVALID, NCORES)
    M[:, pad_mask] = -np.inf

    win = np.argmax(M, axis=1)
    core, blk = win // NBT, win % NBT
    cand = core[:, None] * VS + blk[:, None] * W + np.arange(W)[None, :]  # [BS,W]

    dblk = np.einsum('rd,rwd->rw', er, vocn[cand])
    k = np.argmax(dblk, axis=1)
    jmax = cand[np.arange(BS), k]

    cosmax = (predsN * vocn[jmax]).sum(axis=1) / n
    costgt = (predsN * tgtN).sum(axis=1) / (
        np.maximum(np.sqrt((tgtN ** 2).sum(axis=1)), 1e-12) * n)
    diff = np.maximum(np.float32(GAMMA) + cosmax - costgt, 0.0).astype(np.float32)
    mask = tgt_idx != int(np.asarray(pad_id))
    denom = np.float32(mask.sum())
    loss = np.float32(np.where(mask, diff, np.float32(0.0)).sum() / denom)
    return np.asarray(loss, dtype=np.float32)


def run_cores(in_maps, trace=False):
    from concourse.bass_utils import run_bass_kernel_spmd
    nc = get_nc()
    if not nc.is_finalized():
        nc.finalize()
    return run_bass_kernel_spmd(nc, in_maps, list(range(NCORES)), trace=trace)


def kernel(preds, emb_weight, target, pad_id):
    in_maps = make_in_maps(preds, emb_weight, target)
    res = run_cores(in_maps, trace=False)
    return combine(res.results, preds, emb_weight, target, pad_id)


# revision 14
# speedup vs baseline: 1.0111x; 1.0111x over previous
"""MaxMarginLoss Trainium2 kernel (8 NeuronCores, vocab-sharded).

Math (reference):
    out_norm = l2norm(preds^T over D)            [B,S,D]
    voc_norm = l2norm(emb over D)                [V,D]
    tgt      = emb[target]                       [B,S,D]
    d        = out_norm@voc_norm.T - tgt@voc_norm.T
    jmax     = argmax_v d
    loss     = mean_masked(relu(g + cos[jmax] - cos[target]))

Key identity: d = (out_norm - tgt) @ voc_norm.T  -> ONE matmul.  Per-row
positive scaling keeps the argmax, so each device computes
    slab[s,v] = (preds[s] - n_s*tgt[s]) . voc_norm[v]   ( = n_s * d[s,v] )
with NO division on device.  The matmul runs in fp8e4m3 (DoubleRow perf
mode, 2 k-subtiles per instruction) accumulating f32 in PSUM.

Device outputs ONLY per-row block maxes (blocks of W=16 vocab columns).
Per 2048-col PSUM half: DVE reduce_max's the first 128 cols straight from
PSUM; the scalar engine copies the other 1920 cols to SBUF as bf16, which
DVE then folds 16->8->4 with tensor_max (2-byte dtype = 2x_1port DVE
speed) and reduce_max's to block maxes.  This balances Act/DVE/PE at
~1.7us per half.  No DRAM slab, no gathers, no argmax scan on device.
(gpsimd is idle: walrus implements TensorTensor mult but not max on Pool,
and Pool cannot read PSUM.)

Host combine picks the winning (core, block) per row from the 8x256 block
maxes, recomputes that block's 16 exact dots in f64-free numpy (33 MFLOP,
same scale as the cos_target dots it already does), resolves the exact
within-block argmax, and finishes the masked-mean loss.
"""

import os
import sys

import numpy as np

for _p in ("/opt/trn_rl_repo", "/root/.axon_site/_ro/trn_rl_repo"):
    if os.path.isdir(_p) and _p not in sys.path:
        sys.path.insert(0, _p)

import concourse.bass as bass
import concourse.bacc as bacc_mod
import concourse.mybir as mybir
from concourse.tile import TileContext

P = 128
B, S, D, V = 4, 512, 512, 32000
BS = B * S                  # 2048 rows
NCORES = 8
VS = V // NCORES            # 4000 vocab rows per core
VSP = 4096                  # padded vocab per core
KC = D // P                 # 4 k-subtiles of the contraction
NT = BS // P                # 16 row tiles
HALF = 2048                 # columns per PSUM half
W = 16                      # block width for block maxes
NBT = VSP // W              # 256 blocks per row tile
NBH = HALF // W             # 128 blocks per half
DVE_COLS = 384              # per-half columns reduced by DVE from PSUM
DVE_NB = DVE_COLS // W      # 24
ACT_COLS = HALF - DVE_COLS  # 1664 copied by Act to bf16, folded by DVE
ACT_NB = ACT_COLS // W      # 104
NVALID = VS // W            # 250 non-pad blocks per core
SCALE_E = 0.125
SCALE_V = 16.0
GAMMA = 0.5

F32 = mybir.dt.float32
BF16 = mybir.dt.bfloat16
F8 = mybir.dt.float8e4

_CACHED = {}


def build_nc():
    nc = bacc_mod.Bacc()

    eT8 = nc.declare_dram_parameter("eT8", [P, KC * BS], F8, isOutput=False)
    voc8 = nc.declare_dram_parameter("voc8", [P, KC * VSP], F8, isOutput=False)
    o_bm = nc.declare_dram_parameter("o_bm", [P, NT * NBT], F32, isOutput=True)

    with TileContext(nc) as tc:
        with (
            tc.tile_pool(name="const", bufs=1) as cpool,
            tc.tile_pool(name="stage", bufs=3) as stage,
            tc.tile_pool(name="foldp", bufs=3) as foldp,
            tc.tile_pool(name="bmp", bufs=3) as bmp,
            tc.tile_pool(name="psp", bufs=2, space="PSUM") as psp,
        ):
            eSB = cpool.tile([P, KC * BS], F8, tag="eSB")
            vSB = cpool.tile([P, KC * VSP], F8, tag="vSB")

            # Input loads.  eT is packed row-tile-major and voc chunk-major
            # on the host, so each DMA below is a fully contiguous block and
            # the first matmul only gates on tile0's 512B/partition + the
            # first voc chunk.  Issued in consumption order.
            nc.sync.dma_start(eSB[:, 0:512], eT8[:, 0:512])
            qeng = [nc.scalar, nc.gpsimd]
            for j in range(8):
                js = slice(j * KC * 512, (j + 1) * KC * 512)
                qeng[j % 2].dma_start(vSB[:, js], voc8[:, js])
            nc.sync.dma_start(eSB[:, 512:2048], eT8[:, 512:2048])
            nc.sync.dma_start(eSB[:, 2048:KC * BS], eT8[:, 2048:KC * BS])

            # PE warm-up burst (p-state ramp) while the input DMAs fly;
            # reads uninitialized SBUF, result never consumed.
            w0 = cpool.tile([P, 256], F8, tag="w0")
            x0 = cpool.tile([P, 1024], F8, tag="x0")
            nc.gpsimd.memset(w0, 0.0)
            nc.gpsimd.memset(x0, 0.0)
            psw = psp.tile([P, 2048], F32, tag="ps", name="ps_warm")
            for i in range(4):
                nc.tensor.matmul(
                    psw[:, :512],
                    lhsT=w0[:].rearrange("p (k m) -> p k m", k=2),
                    rhs=x0[:].rearrange("p (k m) -> p k m", k=2),
                    start=True, stop=True,
                    perf_mode=mybir.MatmulPerfMode.DoubleRow,
                )

            # [p, t, k, m]: lhsT slices are contiguous 256B runs
            ev = eSB[:].rearrange("p (t k m) -> p t k m", t=NT, k=KC)
            # [p, j, k, m]: rhs slices are contiguous 1KB runs
            vv = vSB[:].rearrange("p (j k m) -> p j k m", j=8, k=KC)

            for t in range(NT):
                bmt = bmp.tile([P, NBT], F32, tag="bmt")
                for h in range(2):
                    ps = psp.tile([P, HALF], F32, tag="ps")
                    for c in range(4):
                        cs = slice(c * 512, (c + 1) * 512)
                        for q in range(2):
                            nc.tensor.matmul(
                                ps[:, cs],
                                lhsT=ev[:, t, 2 * q:2 * q + 2, :],
                                rhs=vv[:, h * 4 + c, 2 * q:2 * q + 2, :],
                                start=(q == 0), stop=(q == 1),
                                perf_mode=mybir.MatmulPerfMode.DoubleRow,
                            )
                    # DVE: block maxes of the first 128 cols, straight from PSUM
                    nc.vector.reduce_max(
                        bmt[:, h * NBH:h * NBH + DVE_NB],
                        ps[:, 0:DVE_COLS].rearrange("p (b w) -> p b w", w=W),
                        axis=mybir.AxisListType.X,
                    )
                    # Act: bf16-copy remaining 1920 cols; DVE folds 16->8->4
                    # with tensor_max at 2x_1port, then reduce_max's to bm.
                    stg = stage.tile([P, ACT_COLS], BF16, tag="stg")
                    nc.scalar.copy(stg, ps[:, DVE_COLS:HALF])
                    sv = stg[:].rearrange("p (b w) -> p b w", w=W)
                    fs = foldp.tile([P, ACT_NB * 12], BF16, tag="fs")
                    a8 = fs[:, 0:ACT_NB * 8].rearrange("p (b w) -> p b w", w=8)
                    a4 = fs[:, ACT_NB * 8:ACT_NB * 12].rearrange(
                        "p (b w) -> p b w", w=4)
                    nc.vector.tensor_max(a8, sv[:, :, 0:8], sv[:, :, 8:16])
                    nc.vector.tensor_max(a4, a8[:, :, 0:4], a8[:, :, 4:8])
                    nc.vector.reduce_max(
                        bmt[:, h * NBH + DVE_NB:(h + 1) * NBH], a4,
                        axis=mybir.AxisListType.X)
                nc.sync.dma_start(o_bm[:, t * NBT:(t + 1) * NBT], bmt)

    return nc


def get_nc():
    if "nc" not in _CACHED:
        _CACHED["nc"] = build_nc()
    return _CACHED["nc"]


def _prep(preds, emb_weight, target):
    preds = np.ascontiguousarray(np.asarray(preds, dtype=np.float32))     # [B,D,S]
    emb = np.ascontiguousarray(np.asarray(emb_weight, dtype=np.float32))  # [V,D]
    tgt_idx = np.asarray(target).astype(np.int64).reshape(-1)             # [BS]

    predsN = np.ascontiguousarray(preds.transpose(0, 2, 1).reshape(BS, D))
    n = np.maximum(np.sqrt((predsN ** 2).sum(axis=1)), 1e-12).astype(np.float32)
    tgtN = emb[tgt_idx]                                                   # [BS,D]
    er = predsN - n[:, None] * tgtN                                       # [BS,D]
    vocn = emb / np.maximum(
        np.sqrt((emb ** 2).sum(axis=1, keepdims=True)), 1e-12)            # [V,D]
    return predsN, n, tgtN, er, vocn, tgt_idx


def make_in_maps(preds, emb_weight, target):
    import ml_dtypes
    _, _, _, er, vocn, _ = _prep(preds, emb_weight, target)

    e8 = ((er.T) * SCALE_E).astype(ml_dtypes.float8_e4m3)                 # [D,BS]
    # [p, t, k, m]: row-tile-major so per-tile DMA slices are contiguous
    eT8 = np.ascontiguousarray(
        e8.reshape(KC, P, NT, P).transpose(1, 2, 0, 3).reshape(P, KC * BS))

    in_maps = []
    for c in range(NCORES):
        sh = np.zeros((VSP, D), np.float32)
        sh[:VS] = vocn[c * VS:(c + 1) * VS]
        v8 = (sh.T * SCALE_V).astype(ml_dtypes.float8_e4m3)               # [D,VSP]
        # [p, j, k, m]: chunk-major so per-chunk DMA slices are contiguous
        voc8 = np.ascontiguousarray(
            v8.reshape(KC, P, 8, 512).transpose(1, 2, 0, 3).reshape(P, KC * VSP))
        in_maps.append({"eT8": eT8, "voc8": voc8})
    return in_maps


def combine(results, preds, emb_weight, target, pad_id):
    predsN, n, tgtN, er, vocn, tgt_idx = _prep(preds, emb_weight, target)

    # [8, P, NT*NBT] -> M[row, core*NBT + block], row j = t*128 + p
    bm = np.stack([np.asarray(r["o_bm"]) for r in results])
    M = bm.reshape(NCORES, P, NT, NBT).transpose(2, 1, 0, 3).reshape(
        BS, NCORES * NBT)
    pad_mask = np.tile(np.arange(NBT) >= NVALID, NCORES)
    M[:, pad_mask] = -np.inf

    win = np.argmax(M, axis=1)
    core, blk = win // NBT, win % NBT
    cand = core[:, None] * VS + blk[:, None] * W + np.arange(W)[None, :]  # [BS,W]

    dblk = np.einsum('rd,rwd->rw', er, vocn[cand])
    k = np.argmax(dblk, axis=1)
    jmax = cand[np.arange(BS), k]

    cosmax = (predsN * vocn[jmax]).sum(axis=1) / n
    costgt = (predsN * tgtN).sum(axis=1) / (
        np.maximum(np.sqrt((tgtN ** 2).sum(axis=1)), 1e-12) * n)
    diff = np.maximum(np.float32(GAMMA) + cosmax - costgt, 0.0).astype(np.float32)
    mask = tgt_idx != int(np.asarray(pad_id))
    denom = np.float32(mask.sum())
    loss = np.float32(np.where(mask, diff, np.float32(0.0)).sum() / denom)
    return np.asarray(loss, dtype=np.float32)


def run_cores(in_maps, trace=False):
    from concourse.bass_utils import run_bass_kernel_spmd
    nc = get_nc()
    if not nc.is_finalized():
        nc.finalize()
    return run_bass_kernel_spmd(nc, in_maps, list(range(NCORES)), trace=trace)


def kernel(preds, emb_weight, target, pad_id):
    in_maps = make_in_maps(preds, emb_weight, target)
    res = run_cores(in_maps, trace=False)
    return combine(res.results, preds, emb_weight, target, pad_id)
